# revision 8
# baseline (speedup 1.0000x reference)
"""Group Residual VQ kernel for Trainium2 (Bass/Tile), 8 NeuronCores.

Problem: nn_GroupResidualVectorQuantizer
  x [65536, 256] fp32, 4 modalities of 16384 tokens each (equal segments),
  L=4 residual layers, per (layer, modality) codebook = concat(shared[512],
  specific[256]) -> [768, 256].

Sharding: data-parallel over tokens. Core c handles tokens
[c*8192, (c+1)*8192) = half of modality c//2. Codebooks replicated
(each core only needs its own modality's 4 layer codebooks).

Per-core device algorithm (residual kept TRANSPOSED in SBUF, [256, 8192]
as two 128-partition chunks inside one blob):
  for l in 0..3:  for t in 0..63 (token tiles of 128):
    P[n,k]    = sum_d rT[d,n]*cbT[d,k]            (TensorE fp32, PSUM)
    d2'[n,k]  = -2*(P - cbnorm[k]/2)              (VectorE tensor_tensor_reduce,
                + row-min m'[n] in the same pass)
    idx[n]    = argmin_k d2'                      (VectorE max_index search)
    xq        = cb[idx]  (indirect-DMA row gather from HBM codebook)
    rT       -= xq^T     (DVE relay copy, TensorE transpose, VectorE subtract)
The per-row norm term rnorm[n] of the true distance d2 = rnorm - 2P + cbn
is a constant shift per row: argmin doesn't need it, and the host adds it
back (rn_0 = ||x||^2, rn_{l+1} = rn_l + m'_l since min d2 = ||next
residual||^2). Losses also reduce from m' on the host. x_q = x - residual.

All SBUF constants + the residual arrive via ONE input DMA (single DMA
semaphore) and the gathered codeword is relayed through a VectorE copy, so
PE instructions (fp32 matmul = fused-LDWEIGHTS, 1 sync-wait slot) never
need more than one semaphore wait.
"""

import sys

if "/opt/trn_rl_repo" not in sys.path:
    sys.path.insert(0, "/opt/trn_rl_repo")

import numpy as np

L = 4
M = 4
KS = 512
KP = 256
K = KS + KP  # 768
D = 256
N = 65536
NCORES = 8
NPC = N // NCORES  # 8192 tokens per core
T = NPC // 128  # 64 tiles of 128 tokens
BETA = 0.25

# blob free-dim layout (fp32 elements per partition)
OFF_RT0 = 0
OFF_RT1 = NPC
OFF_CBT = 2 * NPC  # 8 chunks of 768: (l*2+i)
OFF_CBN = OFF_CBT + 8 * K  # 4 chunks of 768
OFF_ID = OFF_CBN + 4 * K  # identity [128, 128]
BLOB_F = OFF_ID + 128

_CACHE = {}


def _build_program():
    from concourse import bacc, bass, mybir, tile

    dt = mybir.dt
    nc = bacc.Bacc(
        "TRN2", target_bir_lowering=False, debug=False, num_devices=NCORES
    )

    blob_in = nc.dram_tensor("blob", [128, BLOB_F], dt.float32, kind="ExternalInput")
    cb_dram = [
        nc.dram_tensor(f"cb{l}", [K, D], dt.float32, kind="ExternalInput")
        for l in range(L)
    ]

    d2_out = nc.dram_tensor("d2", [NPC, L, K], dt.float32, kind="ExternalOutput")
    res_out = nc.dram_tensor("resT", [2 * 128, NPC], dt.float32, kind="ExternalOutput")
    idx_out = nc.dram_tensor("idx", [L * 128, T * 8], dt.uint32, kind="ExternalOutput")
    mv_out = nc.dram_tensor("minv", [L * 128, T], dt.float32, kind="ExternalOutput")

    blob_ap = blob_in.ap()
    d2_ap = d2_out.ap()
    res_ap = res_out.ap()
    idx_ap = idx_out.ap()
    mv_ap = mv_out.ap()

    with tile.TileContext(nc) as tc:
        with (
            tc.tile_pool(name="const", bufs=1) as cpool,
            tc.tile_pool(name="work", bufs=4) as wpool,
            tc.tile_pool(name="mm", bufs=2, space="PSUM") as ppool,
            tc.tile_pool(name="tp", bufs=3, space="PSUM") as tpool,
        ):
            blob = cpool.tile([128, BLOB_F], dt.float32, tag="blob", name="blob")
            nc.sync.dma_start(out=blob[:], in_=blob_ap[:, :])

            rt = [blob[:, OFF_RT0 : OFF_RT0 + NPC], blob[:, OFF_RT1 : OFF_RT1 + NPC]]
            cbT = [
                [
                    blob[:, OFF_CBT + (l * 2 + i) * K : OFF_CBT + (l * 2 + i + 1) * K]
                    for i in range(2)
                ]
                for l in range(L)
            ]
            cbn = [blob[:, OFF_CBN + l * K : OFF_CBN + (l + 1) * K] for l in range(L)]
            ident = blob[:, OFF_ID : OFF_ID + 128]

            mv = [
                cpool.tile([128, T], dt.float32, tag=f"mv{l}", name=f"mv{l}")
                for l in range(L)
            ]
            idx = [
                cpool.tile([128, T * 8], dt.uint32, tag=f"idx{l}", name=f"idx{l}")
                for l in range(L)
            ]

            for l in range(L):
                for t in range(T):
                    ts = slice(t * 128, (t + 1) * 128)
                    ps = ppool.tile([128, K], dt.float32, tag="ps", name="ps")
                    # P[n, k] = sum_d rT[d, n] * cbT[d, k], fp32, two k-banks
                    for k0, k1 in ((0, 512), (512, K)):
                        for i in range(2):
                            nc.tensor.matmul(
                                out=ps[:, k0:k1],
                                lhsT=rt[i][:, ts],
                                rhs=cbT[l][i][:, k0:k1],
                                start=(i == 0),
                                stop=(i == 1),
                            )
                    # d2' = -2P + cbn ; m' = min_k d2'
                    d2t = wpool.tile([128, K], dt.float32, tag="d2", name="d2t")
                    nc.scalar.mul(out=d2t[:], in_=ps[:], mul=-2.0)  # ACT, PSUM->SBUF
                    nc.vector.tensor_add(out=d2t[:], in0=d2t[:], in1=cbn[l][:])
                    nc.sync.dma_start(
                        out=d2_ap[ts, l : l + 1, :], in_=d2t[:, None, :]
                    )
                    nc.vector.tensor_reduce(
                        out=mv[l][:, t : t + 1],
                        in_=d2t[:],
                        axis=mybir.AxisListType.X,
                        op=mybir.AluOpType.min,
                    )
                    # argmin: search for the min value's position in the row
                    mb8 = wpool.tile([128, 8], dt.float32, tag="mb8", name="mb8")
                    nc.scalar.copy(
                        out=mb8[:], in_=mv[l][:, t : t + 1].to_broadcast([128, 8])
                    )
                    nc.vector.max_index(
                        out=idx[l][:, t * 8 : (t + 1) * 8],
                        in_max=mb8[:],
                        in_values=d2t[:],
                    )
                    # xq = cb[idx] : gather 128 rows of 1KB from HBM
                    xq = wpool.tile([128, D], dt.float32, tag="xq", name="xq")
                    nc.gpsimd.indirect_dma_start(
                        out=xq[:],
                        out_offset=None,
                        in_=cb_dram[l].ap(),
                        in_offset=bass.IndirectOffsetOnAxis(
                            ap=idx[l][:, t * 8 : t * 8 + 1], axis=0
                        ),
                    )
                    # rT -= xq^T  (PE transpose, ACT PSUM->SBUF, GpSimd sub)
                    xqT = tpool.tile([128, 256], dt.float32, tag="xqT", name="xqT")
                    nc.tensor.transpose(
                        out=xqT[:, 0:128], in_=xq[:, 0:128], identity=ident
                    )
                    nc.tensor.transpose(
                        out=xqT[:, 128:256], in_=xq[:, 128:256], identity=ident
                    )
                    xqTs = wpool.tile([128, 256], dt.float32, tag="xqTs", name="xqTs")
                    nc.scalar.copy(out=xqTs[:], in_=xqT[:])
                    for i in range(2):
                        nc.gpsimd.tensor_sub(
                            out=rt[i][:, ts],
                            in0=rt[i][:, ts],
                            in1=xqTs[:, i * 128 : (i + 1) * 128],
                        )

            for i in range(2):
                nc.sync.dma_start(
                    out=res_ap[i * 128 : (i + 1) * 128, :], in_=rt[i][:]
                )
            for l in range(L):
                nc.sync.dma_start(out=idx_ap[l * 128 : (l + 1) * 128, :], in_=idx[l][:])
                nc.sync.dma_start(out=mv_ap[l * 128 : (l + 1) * 128, :], in_=mv[l][:])

    nc.compile()
    return nc


def _get_nc():
    if "nc" not in _CACHE:
        _CACHE["nc"] = _build_program()
    return _CACHE["nc"]


def _make_in_maps(x, share_emb, specific_emb):
    x = np.ascontiguousarray(x, dtype=np.float32)
    share = np.asarray(share_emb, dtype=np.float32)
    spec = np.asarray(specific_emb, dtype=np.float32)

    cb = np.empty((L, M, K, D), dtype=np.float32)
    cb[:, :, :KS, :] = share[:, None, :, :]
    cb[:, :, KS:, :] = spec
    cbT = np.ascontiguousarray(np.swapaxes(cb, 2, 3))  # [L, M, D, K]
    mhcbn = np.einsum("lmkd,lmkd->lmk", cb, cb).astype(np.float32)

    in_maps = []
    for c in range(NCORES):
        m = c // 2
        sl = slice(c * NPC, (c + 1) * NPC)
        xT = x[sl].T  # [256, 8192] view
        blob = np.empty((128, BLOB_F), dtype=np.float32)
        blob[:, OFF_RT0 : OFF_RT0 + NPC] = xT[:128]
        blob[:, OFF_RT1 : OFF_RT1 + NPC] = xT[128:]
        for l in range(L):
            for i in range(2):
                blob[:, OFF_CBT + (l * 2 + i) * K : OFF_CBT + (l * 2 + i + 1) * K] = (
                    cbT[l, m, i * 128 : (i + 1) * 128, :]
                )
            blob[:, OFF_CBN + l * K : OFF_CBN + (l + 1) * K] = mhcbn[l, m][None, :]
        blob[:, OFF_ID : OFF_ID + 128] = np.eye(128, dtype=np.float32)
        im = {"blob": blob}
        for l in range(L):
            im[f"cb{l}"] = np.ascontiguousarray(cb[l, m])
        in_maps.append(im)
    return in_maps


def _run_device(x, share_emb, specific_emb, trace=False):
    from concourse.bass_utils import run_bass_kernel_spmd

    nc = _get_nc()
    in_maps = _make_in_maps(x, share_emb, specific_emb)
    res = run_bass_kernel_spmd(nc, in_maps, core_ids=list(range(NCORES)), trace=trace)
    return res


def _assemble(x, res_list):
    x = np.asarray(x, dtype=np.float32)
    xsq = np.einsum("nd,nd->n", x, x).astype(np.float32)

    residual = np.empty((N, D), dtype=np.float32)
    distances = np.empty((N, L, K), dtype=np.float32)
    indices = np.empty((N, L), dtype=np.int32)
    q_losses = np.zeros((L, M), dtype=np.float64)

    for c in range(NCORES):
        out = res_list[c]
        sl = slice(c * NPC, (c + 1) * NPC)
        residual[sl] = out["resT"].reshape(D, NPC).T
        idx = out["idx"].reshape(L, 128, T, 8)[:, :, :, 0]  # [L, 128, T]
        indices[sl] = idx.transpose(2, 1, 0).reshape(NPC, L).astype(np.int32)

        # rnorm chain: rn_0 = ||x||^2, rn_{l+1} = rn_l + m'_l
        mvtok = out["minv"].reshape(L, 128, T).transpose(0, 2, 1).reshape(L, NPC)
        rn = np.empty((L, NPC), dtype=np.float32)
        rn[0] = xsq[sl]
        for l in range(1, L):
            rn[l] = rn[l - 1] + mvtok[l - 1]
        # true d2 = d2' + rn (per row, per layer)
        distances[sl] = out["d2"] + rn.T[:, :, None]
        q_losses[:, c // 2] += (mvtok + rn).sum(axis=1, dtype=np.float64)

    q_losses = ((1.0 + BETA) * q_losses / float((N // M) * D)).astype(np.float32)
    x_q = x - residual
    return x_q, residual, indices, distances, q_losses


def kernel(x, split_index=None, share_emb=None, specific_emb=None):
    res = _run_device(x, share_emb, specific_emb, trace=False)
    return _assemble(x, res.results)
